# revision 37
# baseline (speedup 1.0000x reference)
"""Trainium2 Bass kernel for the binarized BasicBlock (dense_cnn) — v3.

Contract: kernel(**inputs) takes the FULL unsharded inputs (numpy arrays,
keyed as in reference.setup_inputs()) and returns the FULL output
(32, 128, 56, 56) float32.  Internally shards the batch dim across 8
NeuronCores (pure data parallel, params replicated).

v3 design (memory-regime: halve HBM traffic, rebalance engines):
 - x is shipped fp16 in a host-packed slab layout [pair, half, 128, 57*112]
   so each unit load is one fully-contiguous [128, 6384] DMA.  Output is
   stored fp16 and upcast on host.  HBM traffic drops 19.3MB -> 9.8MB/core.
 - sign1 runs entirely on DVE as a u16 bit trick (fp16 sign bit -> fp16 +-1),
   eligible for the 4x perf mode (16-bit, step 1, 4B-aligned, single-src).
 - the avgpool shortcut is fused into the conv psum as 4 identity taps
   (weight d0 = fp16(1/(4*s3))) on the raw fp16 x slab; prelu1 reads psum
   directly with scale=s3 (fp32).  No DVE rowsum/colsum/merge at all.
 - conv1 uses 2x2 PE quadrant packing: per tap, 4 concurrent 64x64 matmuls
   (img A/B x chunk c/c+1) with chunk parity swapping the psum partition
   half; all downstream ops are parity-agnostic (params identical per
   partition half) except stage2's rhs slicing, which follows the parity.
 - sign2 = u16 bit trick on fp16 out1 (valid since b13+b21==0 and a1>0).
 - stage2 per chunk: pw + diag matmuls into a [128,1024] psum pair tile,
   one strided Prelu over both images, fp16 stores per half-unit.
"""
import sys

sys.path.insert(0, "/opt/trn_rl_repo")

import numpy as np

import concourse.bacc as bacc
import concourse.mybir as mybir
import concourse.tile as tile
from concourse import bass_utils

# Problem shapes (hardcoded per spec)
B, CIN, H, W = 32, 64, 112, 112
COUT = 2 * CIN
NCORES = 8
BPC = B // NCORES          # images per core = 4
NPAIR = BPC // 2           # image pairs per core = 2
OH, OW = H // 2, W // 2    # 56, 56
HALF = OH // 2             # 28 output rows per unit
NCHUNK = 4                 # psum chunks per unit (7 out rows each)
CROWS = HALF // NCHUNK     # 7
CN = CROWS * OW            # 392 cols per chunk
UN = HALF * OW             # 1568 elems per unit (per partition)
SROWS = 57                 # slab rows (input rows 2*oy0-1 .. 2*oy0+55)
SPITCH = 114               # sign slab col pitch (2 pad cols, signs at 2:114)

# param columns
PA1, PB12, PB11, PA2F, PB22F, PS2V, PBS2, PB13, PB23F, PA1M = range(10)
NPARAM = 10
# weight blocks of 64 cols: conv taps 0..8 (ky*3+kx), identity d0 block,
# then two 128-wide blocks: [wpw1|wpw2] and [diag1|diag2]
O_ID = 9 * 64              # identity (avgpool) block
O_PW = 10 * 64             # [wpw1|wpw2]
O_DIAG = 10 * 64 + 128     # [diag1|diag2]
WCOLS = 10 * 64 + 256

_cache = {}


def _build(scal, reps=1):
    nc = bacc.Bacc("TRN2", target_bir_lowering=False, debug=False)
    f32 = mybir.dt.float32
    f16 = mybir.dt.float16
    u16 = mybir.dt.uint16
    AF = mybir.ActivationFunctionType
    ALU = mybir.AluOpType

    s3f = scal["s3"]
    b11_zero = scal["b11_zero"]
    trick_sign2 = scal["trick_sign2"]
    has_b13 = scal["has_b13"]
    has_b23 = scal["has_b23"]

    tc_cm = tile.TileContext(nc)
    tc = tc_cm.__enter__()
    dram_cm = tc.tile_pool(name="dram", bufs=1, space="DRAM")
    dram = dram_cm.__enter__()

    x_d = dram.tile([NPAIR, 2, 128, SROWS * W], f16, kind="ExternalInput")
    w_d = dram.tile([128, WCOLS], f16, kind="ExternalInput")
    p_d = dram.tile([128, NPARAM], f32, kind="ExternalInput")
    # pair-major output: [pair, ch, img-in-pair, pix]; host transposes back
    y_d = dram.tile([NPAIR, COUT, 2, OH * OW], f16, kind="ExternalOutput")

    pools = []

    def pool(name, **kw):
        cm = tc.tile_pool(name=name, **kw)
        pools.append(cm)
        return cm.__enter__()

    const = pool("const", bufs=1)
    pers = pool("pers", bufs=1)
    slab = pool("slab", bufs=4)
    work = pool("work", bufs=2)
    psum = pool("psum", bufs=2, space="PSUM")
    psum2 = pool("psum2", bufs=2, space="PSUM")

    wt = const.tile([128, WCOLS], f16)
    pt = const.tile([128, NPARAM], f32)

    # persistent sign slabs indexed by half h; cols 0:2 are permanent zero
    # pads (col 1 = input col -1), and for h=0 row 0 is the zero pad row.
    sp = [pers.tile([128, SROWS * SPITCH], f16, tag=f"sp{h}", name=f"sp{h}")
          for h in range(2)]
    for h in range(2):
        spv0 = sp[h][:].rearrange("p (r c) -> p r c", r=SROWS)
        nc.vector.memset(spv0[:, :, 0:2], 0.0)
    nc.vector.memset(
        sp[0][:].rearrange("p (r c) -> p r c", r=SROWS)[:, 0:1, :], 0.0)

    units = [(p, h) for _ in range(reps)
             for p in range(NPAIR) for h in range(2)]
    xps = {}
    signed = set()

    def emit_load(k, gate=None):
        if k >= len(units) or k in xps:
            return
        p, h = units[k]
        xp = slab.tile([128, SROWS * W], f16, tag="xp", name=f"xp{k}")
        ld0 = 1 if h == 0 else 0
        if gate is not None:
            # WAR gate: a 1-elem DVE copy reading the just-signed sp makes
            # this DMA wait for that sign1, keeping late units' loads out
            # of the SDMA round-robin while the early slabs stream
            nc.vector.tensor_copy(xp[:, ld0 * W:ld0 * W + 1], gate)
        # unit 0 split in two so q0's conv can start early; rest one DMA
        bands = [(ld0, 29), (29, SROWS)] if k == 0 else [(ld0, SROWS)]
        for (ra, rb) in bands:
            nc.sync.dma_start(
                xp[:, ra * W:rb * W], x_d[p, h, :, ra * W:rb * W])
        xps[k] = (xp, bands)

    def emit_sign(k):
        """sign1 bit trick for unit k: fp16 x -> fp16 +-1 in sp[h]."""
        if k >= len(units) or k in signed:
            return
        signed.add(k)
        p, h = units[k]
        xp, bands = xps[k]
        xpv = xp[:].rearrange("p (r c) -> p r c", r=SROWS)
        spv = sp[h][:].rearrange("p (r c) -> p r c", r=SROWS)
        for (ra, rb) in bands:
            if b11_zero:
                nc.vector.tensor_scalar(
                    spv[:, ra:rb, 2:114].bitcast(u16),
                    xpv[:, ra:rb, :].bitcast(u16), 0x8000, 0x3C00,
                    ALU.bitwise_and, ALU.bitwise_or)
            else:
                nc.scalar.activation(
                    spv[:, ra:rb, 2:114], xpv[:, ra:rb, :],
                    AF.Sign, bias=pt[:, PB11:PB11 + 1])
        if k == 0:
            # u2/u3 loads start once unit 0 is signed: the early window
            # belongs to u0's bands + wt/pt + u1
            g = sp[h][:, bands[-1][0] * SPITCH + 2: bands[-1][0] * SPITCH + 3]
            emit_load(2, gate=g)
            emit_load(3, gate=g)

    # u0's slab and wt/pt first, u1 racing behind; u2/u3 gated on sign1(u0)
    emit_load(0)
    nc.sync.dma_start(wt[:], w_d[:])
    nc.sync.dma_start(pt[:], p_d[:])
    emit_load(1)
    emit_sign(0)

    def conv_mm(cp, spv, xpv, t, c, cc, start, stop):
        """One tap MM for chunk c. Parity cc swaps psum halves."""
        for i in range(2):          # i: img A/B (rhs partition half)
            rp = slice(64 * i, 64 * i + 64)
            ob = 64 * ((i + cc) % 2)  # psum partition half (parity swap)
            op = slice(ob, ob + 64)
            if t < 9:
                ky, kx = divmod(t, 3)
                rhs = spv[rp, ky + 14 * c: ky + 14 * c + 13: 2,
                          1 + kx: 1 + kx + 111: 2]
                w = wt[rp, 64 * t:64 * t + 64]
            else:
                dy, dx = divmod(t - 9, 2)
                rhs = xpv[rp, 1 + 14 * c + dy: 1 + 14 * c + dy + 13: 2,
                          dx: dx + 111: 2]
                w = wt[rp, O_ID:O_ID + 64]
            nc.tensor.matmul(
                cp[op, 512 * cc:512 * cc + CN], w, rhs,
                start=start, stop=stop)

    pending = []   # deferred stage2 emitters from the previous unit

    for k, (p, h) in enumerate(units):
        nA, nB = 2 * p, 2 * p + 1
        oy0 = HALF * h
        xp, _ = xps[k]
        xpv = xp[:].rearrange("p (r c) -> p r c", r=SROWS)
        spv = sp[h][:].rearrange("p (r c) -> p r c", r=SROWS)

        out1 = work.tile([128, UN], f16, tag="out1", name="out1")
        sg2 = work.tile([128, UN], f16, tag="sg2", name="sg2")
        stg = work.tile([128, 2 * UN], f16, tag="stg", name="stg")

        # ---- conv1 + fused avgpool ----
        # units 1+: t-major over the full unit (8 MMs/tap across both psum
        # tiles) so a psum region is revisited every 8 MMs and the
        # accumulation RAW drain hides (185ns/4-MM group); unit 0 runs
        # q-split so its first taps only need slab rows <29.  stage2
        # chunks of unit k-1 interleave at tap boundaries.
        cp_list = [psum.tile([128, 1024], f32, tag="ps", name=f"ps{k}_{q}")
                   for q in range(2)]

        def prelu1_dve(q):
            # 3-op DVE prelu: first op frees the conv psum tile in ~940ns
            # without queueing behind the busy ACT engine
            hs = slice(2 * CN * q, 2 * CN * (q + 1))
            pin = cp_list[q][:].rearrange("p (i n) -> p i n", i=2)[:, :, 0:CN]
            t16 = work.tile([128, 2 * CN], f16, tag="t16", name="t16")
            tv = t16[:].rearrange("p (i n) -> p i n", i=2)
            nc.vector.tensor_scalar(
                tv, pin, s3f, pt[:, PB12:PB12 + 1], ALU.mult, ALU.add)
            nc.vector.tensor_scalar(
                sg2[:, hs].bitcast(u16), t16[:].bitcast(u16),
                0x8000, 0x3C00, ALU.bitwise_and, ALU.bitwise_or)
            m16 = work.tile([128, 2 * CN], f16, tag="m16", name="m16")
            nc.vector.tensor_scalar(
                m16[:], t16[:], 0.0, pt[:, PA1M:PA1M + 1],
                ALU.min, ALU.mult)
            nc.vector.tensor_tensor(
                out1[:, hs], t16[:], m16[:], ALU.add)

        def prelu1_act(q):
            hs = slice(2 * CN * q, 2 * CN * (q + 1))
            pin = cp_list[q][:].rearrange("p (i n) -> p i n", i=2)[:, :, 0:CN]
            pout = out1[:, hs].rearrange("p (i n) -> p i n", i=2)
            nc.scalar.activation(
                pout, pin, AF.Prelu,
                bias=pt[:, PB12:PB12 + 1], scale=s3f,
                alpha=pt[:, PA1:PA1 + 1])
            if has_b13:
                nc.vector.tensor_scalar(
                    out1[:, hs], out1[:, hs], pt[:, PB13:PB13 + 1],
                    None, ALU.add)
            if trick_sign2:
                nc.vector.tensor_scalar(
                    sg2[:, hs].bitcast(u16), out1[:, hs].bitcast(u16),
                    0x8000, 0x3C00, ALU.bitwise_and, ALU.bitwise_or)
            else:
                nc.scalar.activation(
                    sg2[:, hs], out1[:, hs], AF.Sign,
                    bias=pt[:, PBS2:PBS2 + 1])

        dve_ok = trick_sign2 and not has_b13
        if k == 0:
            for q in range(2):
                for t in range(13):
                    for cc in range(2):
                        conv_mm(cp_list[q], spv, xpv, t, 2 * q + cc, cc,
                                start=(t == 0), stop=(t == 12))
                if q == 1 and dve_ok:
                    prelu1_dve(1)
                else:
                    prelu1_act(q)
                if q == 0:
                    emit_sign(k + 1)
        else:
            for t in range(13):
                for q in range(2):
                    for cc in range(2):
                        conv_mm(cp_list[q], spv, xpv, t, 2 * q + cc, cc,
                                start=(t == 0), stop=(t == 12))
                if t in (4, 9) and pending:
                    pending.pop(0)()
            if dve_ok:
                prelu1_dve(1)
            else:
                prelu1_act(1)
            prelu1_act(0)
            emit_sign(k + 1)

        while pending:
            pending.pop(0)()

        # ---- stage 2 (deferred into unit k+1's conv window) ----
        def mk_stage2(c, k=k, out1=out1, sg2=sg2, stg=stg,
                      nA=nA, nB=nB, oy0=oy0):
            def emit(pool2=psum2, tag2="ps2"):
                cs = slice(CN * c, CN * (c + 1))
                p2 = pool2.tile([128, 1024], f32, tag=tag2, name="ps2")
                # slot 0 (cols 0:CN) = img A, slot 1 (512:) = img B;
                # chunk parity decides which sbuf partition half holds A
                for blk, src, st, sp_ in ((O_PW, sg2, True, False),
                                          (O_DIAG, out1, False, True)):
                    for i in range(2):       # i: img A/B (psum slot)
                        rb = 64 * ((i + c) % 2)
                        rp = slice(rb, rb + 64)
                        nc.tensor.matmul(
                            p2[:, 512 * i:512 * i + CN],
                            wt[rp, blk:blk + 128], src[rp, cs],
                            start=st, stop=sp_)
                pin = p2[:].rearrange("p (i n) -> p i n", i=2)[:, :, 0:CN]
                pout = stg[:].rearrange("p (i n) -> p i n", i=2)[:, :, cs]
                nc.scalar.activation(
                    pout, pin, AF.Prelu,
                    bias=pt[:, PB22F:PB22F + 1],
                    scale=pt[:, PS2V:PS2V + 1],
                    alpha=pt[:, PA2F:PA2F + 1])
                if has_b23 and c == NCHUNK - 1:
                    nc.vector.tensor_scalar(
                        stg[:], stg[:], pt[:, PB23F:PB23F + 1],
                        None, ALU.add)
                if has_b23:
                    rr = (0, HALF) if c == NCHUNK - 1 else None
                else:
                    rr = {1: (0, 14), NCHUNK - 1: (14, HALF)}.get(c)
                if rr is not None:
                    # one DMA stores both images (pair-major y_d layout)
                    sv = stg[:].rearrange("p (i n) -> p i n", i=2)
                    nc.sync.dma_start(
                        y_d[nA // 2, :, :,
                            OW * (oy0 + rr[0]):OW * (oy0 + rr[1])],
                        sv[:, :, OW * rr[0]:OW * rr[1]])
            return emit

        pending = [mk_stage2(c) for c in range(NCHUNK)]

    # tail: the conv psum pool is idle now, so alternate the last unit's
    # stage2 chunks between both psum pools — no prelu2 WAR turnaround
    for ci, fn in enumerate(pending):
        if ci % 2 == 1:
            fn(pool2=psum, tag2="ps")
        else:
            fn()
    pending = []

    for cm in reversed(pools):
        cm.__exit__(None, None, None)
    dram_cm.__exit__(None, None, None)
    tc_cm.__exit__(None, None, None)
    nc.compile()
    return nc, x_d.name, w_d.name, p_d.name, y_d.name


def _prep(inputs):
    f32 = np.float32
    f16 = np.float16
    w3 = np.asarray(inputs["w3"], f32)
    wpw1 = np.asarray(inputs["wpw1"], f32)
    wpw2 = np.asarray(inputs["wpw2"], f32)
    a1 = np.asarray(inputs["a1"], f32).reshape(CIN)
    a2 = np.asarray(inputs["a2"], f32).reshape(COUT)
    b11 = np.asarray(inputs["b11"], f32).reshape(CIN)
    b12 = np.asarray(inputs["b12"], f32).reshape(CIN)
    b13 = np.asarray(inputs["b13"], f32).reshape(CIN)
    b21 = np.asarray(inputs["b21"], f32).reshape(CIN)
    b22 = np.asarray(inputs["b22"], f32).reshape(COUT)
    b23 = np.asarray(inputs["b23"], f32).reshape(COUT)

    s3 = f32(np.mean(np.abs(w3))) or f32(1.0)
    s1 = f32(np.mean(np.abs(wpw1))) or f32(1.0)
    s2 = f32(np.mean(np.abs(wpw2))) or f32(1.0)

    d0 = f16(1.0 / (4.0 * float(s3)))
    d1 = f16(1.0 / float(s1))
    d2 = f16(1.0 / float(s2))

    whalf = np.zeros((64, WCOLS), f32)
    sgn = np.sign
    for t in range(9):
        ky, kx = divmod(t, 3)
        whalf[:, 64 * t:64 * t + 64] = sgn(w3[:, :, ky, kx]).T
    whalf[:, O_ID:O_ID + 64] = float(d0) * np.eye(64, dtype=f32)
    whalf[:, O_PW:O_PW + 64] = sgn(wpw1[:, :, 0, 0]).T
    whalf[:, O_PW + 64:O_PW + 128] = sgn(wpw2[:, :, 0, 0]).T
    whalf[:, O_DIAG:O_DIAG + 64] = float(d1) * np.eye(64, dtype=f32)
    whalf[:, O_DIAG + 64:O_DIAG + 128] = float(d2) * np.eye(64, dtype=f32)
    wfull = np.concatenate([whalf, whalf], axis=0).astype(f16)

    def pairc(v):  # channel vec (64,) -> pair-layout (128,)
        return np.concatenate([v, v])

    params = np.zeros((128, NPARAM), f32)
    params[:, PA1] = pairc(a1)
    params[:, PB12] = pairc(b12)
    params[:, PB11] = pairc(b11)
    params[:, PA2F] = a2
    params[:, PB22F] = b22
    params[:, PS2V] = np.concatenate(
        [np.full(64, 1.0 / float(d1), f32), np.full(64, 1.0 / float(d2), f32)])
    params[:, PBS2] = pairc(b13 + b21)
    params[:, PB13] = pairc(b13)
    params[:, PB23F] = b23
    params[:, PA1M] = pairc(a1) - 1.0

    scal = {
        "s3": float(s3),
        "b11_zero": bool(np.all(b11 == 0.0)),
        "trick_sign2": bool(np.all(b13 + b21 == 0.0) and np.all(a1 > 0)),
        "has_b13": bool(np.any(b13 != 0.0)),
        "has_b23": bool(np.any(b23 != 0.0)),
    }
    return wfull, params, scal


def _pack_x(x):
    """x (32,64,112,112) fp32 -> per-core slabs
    [NCORES][NPAIR, 2, 128, 57*112] fp16 (row -1 zero-padded for h=0)."""
    xh = x.astype(np.float16)
    # keep the sign of values that underflow to 0 in fp16 (sign1 must match)
    m = (xh == 0) & (x != 0)
    if m.any():
        xh[m] = np.copysign(np.float16(6e-8), x[m]).astype(np.float16)
    out = np.zeros((NCORES, NPAIR, 2, 2, CIN, SROWS, W), np.float16)
    xc = xh.reshape(NCORES, NPAIR, 2, CIN, H, W)
    for h in range(2):
        r0 = 2 * (HALF * h) - 1
        a = max(r0, 0)
        b = r0 + SROWS
        out[:, :, h, :, :, a - r0:, :] = xc[:, :, :, :, a:b, :]
    # [core, pair, h, img, cin, r, w] -> [core, pair, h, (img cin), r*w]
    return np.ascontiguousarray(
        out.transpose(0, 1, 2, 3, 4, 5, 6)).reshape(
            NCORES, NPAIR, 2, 128, SROWS * W)


def make_in_maps(inputs):
    x = np.asarray(inputs["x"], np.float32)
    wfull, params, scal = _prep(inputs)
    xs = _pack_x(x)
    key = tuple(sorted(scal.items())) + (float(params.sum()),)
    if key not in _cache:
        _cache.clear()
        _cache[key] = _build(scal)
    nc, xn, wn, pn, yn = _cache[key]
    in_maps = [{xn: np.ascontiguousarray(xs[i]), wn: wfull, pn: params}
               for i in range(NCORES)]
    return nc, in_maps, yn


def kernel(**inputs):
    nc, in_maps, yn = make_in_maps(inputs)
    res = bass_utils.run_bass_kernel_spmd(
        nc, in_maps, core_ids=list(range(NCORES)))
    # device output is [pair, ch, img-in-pair, pix]; restore [img, ch, h, w]
    out = np.concatenate(
        [res.results[i][yn].reshape(NPAIR, COUT, 2, OH, OW)
         .transpose(0, 2, 1, 3, 4).reshape(BPC, COUT, OH, OW)
         for i in range(NCORES)], axis=0)
    return out.astype(np.float32)


# revision 38
# speedup vs baseline: 1.1127x; 1.1127x over previous
"""Trainium2 Bass kernel for the binarized BasicBlock (dense_cnn) — v3.

Contract: kernel(**inputs) takes the FULL unsharded inputs (numpy arrays,
keyed as in reference.setup_inputs()) and returns the FULL output
(32, 128, 56, 56) float32.  Internally shards the batch dim across 8
NeuronCores (pure data parallel, params replicated).

v3 design (memory-regime: halve HBM traffic, rebalance engines):
 - x is shipped fp16 in a host-packed slab layout [pair, half, 128, 57*112]
   so each unit load is one fully-contiguous [128, 6384] DMA.  Output is
   stored fp16 and upcast on host.  HBM traffic drops 19.3MB -> 9.8MB/core.
 - sign1 runs entirely on DVE as a u16 bit trick (fp16 sign bit -> fp16 +-1),
   eligible for the 4x perf mode (16-bit, step 1, 4B-aligned, single-src).
 - the avgpool shortcut is fused into the conv psum as 4 identity taps
   (weight d0 = fp16(1/(4*s3))) on the raw fp16 x slab; prelu1 reads psum
   directly with scale=s3 (fp32).  No DVE rowsum/colsum/merge at all.
 - conv1 uses 2x2 PE quadrant packing: per tap, 4 concurrent 64x64 matmuls
   (img A/B x chunk c/c+1) with chunk parity swapping the psum partition
   half; all downstream ops are parity-agnostic (params identical per
   partition half) except stage2's rhs slicing, which follows the parity.
 - sign2 = u16 bit trick on fp16 out1 (valid since b13+b21==0 and a1>0).
 - stage2 per chunk: pw + diag matmuls into a [128,1024] psum pair tile,
   one strided Prelu over both images, fp16 stores per half-unit.
"""
import sys

sys.path.insert(0, "/opt/trn_rl_repo")

import numpy as np

import concourse.bacc as bacc
import concourse.mybir as mybir
import concourse.tile as tile
from concourse import bass_utils

# Problem shapes (hardcoded per spec)
B, CIN, H, W = 32, 64, 112, 112
COUT = 2 * CIN
NCORES = 8
BPC = B // NCORES          # images per core = 4
NPAIR = BPC // 2           # image pairs per core = 2
OH, OW = H // 2, W // 2    # 56, 56
HALF = OH // 2             # 28 output rows per unit
NCHUNK = 4                 # psum chunks per unit (7 out rows each)
CROWS = HALF // NCHUNK     # 7
CN = CROWS * OW            # 392 cols per chunk
UN = HALF * OW             # 1568 elems per unit (per partition)
SROWS = 57                 # slab rows (input rows 2*oy0-1 .. 2*oy0+55)
SPITCH = 114               # sign slab col pitch (2 pad cols, signs at 2:114)

# param columns
PA1, PB12, PB11, PA2F, PB22F, PS2V, PBS2, PB13, PB23F, PA1M = range(10)
NPARAM = 10
# weight blocks of 64 cols: conv taps 0..8 (ky*3+kx), identity d0 block,
# then two 128-wide blocks: [wpw1|wpw2] and [diag1|diag2]
O_ID = 9 * 64              # identity (avgpool) block
O_PW = 10 * 64             # [wpw1|wpw2]
O_DIAG = 10 * 64 + 128     # [diag1|diag2]
WCOLS = 10 * 64 + 256

_cache = {}


def _build(scal, reps=1):
    nc = bacc.Bacc("TRN2", target_bir_lowering=False, debug=False)
    f32 = mybir.dt.float32
    f16 = mybir.dt.float16
    u16 = mybir.dt.uint16
    AF = mybir.ActivationFunctionType
    ALU = mybir.AluOpType

    s3f = scal["s3"]
    b11_zero = scal["b11_zero"]
    trick_sign2 = scal["trick_sign2"]
    has_b13 = scal["has_b13"]
    has_b23 = scal["has_b23"]

    tc_cm = tile.TileContext(nc)
    tc = tc_cm.__enter__()
    dram_cm = tc.tile_pool(name="dram", bufs=1, space="DRAM")
    dram = dram_cm.__enter__()

    x_d = dram.tile([NPAIR, 2, 128, SROWS * W], f16, kind="ExternalInput")
    w_d = dram.tile([128, WCOLS], f16, kind="ExternalInput")
    p_d = dram.tile([128, NPARAM], f32, kind="ExternalInput")
    # pair-major output: [pair, ch, img-in-pair, pix]; host transposes back
    y_d = dram.tile([NPAIR, COUT, 2, OH * OW], f16, kind="ExternalOutput")

    pools = []

    def pool(name, **kw):
        cm = tc.tile_pool(name=name, **kw)
        pools.append(cm)
        return cm.__enter__()

    const = pool("const", bufs=1)
    pers = pool("pers", bufs=1)
    slab = pool("slab", bufs=4)
    work = pool("work", bufs=2)
    psum = pool("psum", bufs=2, space="PSUM")
    psum2 = pool("psum2", bufs=2, space="PSUM")

    wt = const.tile([128, WCOLS], f16)
    pt = const.tile([128, NPARAM], f32)

    # persistent sign slabs indexed by half h; cols 0:2 are permanent zero
    # pads (col 1 = input col -1), and for h=0 row 0 is the zero pad row.
    sp = [pers.tile([128, SROWS * SPITCH], f16, tag=f"sp{h}", name=f"sp{h}")
          for h in range(2)]
    for h in range(2):
        spv0 = sp[h][:].rearrange("p (r c) -> p r c", r=SROWS)
        nc.vector.memset(spv0[:, :, 0:2], 0.0)
    nc.vector.memset(
        sp[0][:].rearrange("p (r c) -> p r c", r=SROWS)[:, 0:1, :], 0.0)

    units = [(p, h) for _ in range(reps)
             for p in range(NPAIR) for h in range(2)]
    xps = {}
    signed = set()

    def emit_load(k, gate=None):
        if k >= len(units) or k in xps:
            return
        p, h = units[k]
        xp = slab.tile([128, SROWS * W], f16, tag="xp", name=f"xp{k}")
        ld0 = 1 if h == 0 else 0
        if gate is not None:
            # WAR gate: a 1-elem DVE copy reading the just-signed sp makes
            # this DMA wait for that sign1, keeping late units' loads out
            # of the SDMA round-robin while the early slabs stream
            nc.vector.tensor_copy(xp[:, ld0 * W:ld0 * W + 1], gate)
        # unit 0 split in two so q0's conv can start early; rest one DMA
        bands = [(ld0, 29), (29, SROWS)] if k == 0 else [(ld0, SROWS)]
        for (ra, rb) in bands:
            nc.sync.dma_start(
                xp[:, ra * W:rb * W], x_d[p, h, :, ra * W:rb * W])
        xps[k] = (xp, bands)

    def emit_sign(k):
        """sign1 bit trick for unit k: fp16 x -> fp16 +-1 in sp[h]."""
        if k >= len(units) or k in signed:
            return
        signed.add(k)
        p, h = units[k]
        xp, bands = xps[k]
        xpv = xp[:].rearrange("p (r c) -> p r c", r=SROWS)
        spv = sp[h][:].rearrange("p (r c) -> p r c", r=SROWS)
        for (ra, rb) in bands:
            if b11_zero:
                nc.vector.tensor_scalar(
                    spv[:, ra:rb, 2:114].bitcast(u16),
                    xpv[:, ra:rb, :].bitcast(u16), 0x8000, 0x3C00,
                    ALU.bitwise_and, ALU.bitwise_or)
            else:
                nc.scalar.activation(
                    spv[:, ra:rb, 2:114], xpv[:, ra:rb, :],
                    AF.Sign, bias=pt[:, PB11:PB11 + 1])
        if k == 0:
            # u2/u3 loads start once unit 0 is signed: the early window
            # belongs to u0's bands + wt/pt + u1
            g = sp[h][:, bands[-1][0] * SPITCH + 2: bands[-1][0] * SPITCH + 3]
            emit_load(2, gate=g)
            emit_load(3, gate=g)

    # u0's slab and wt/pt first, u1 racing behind; u2/u3 gated on sign1(u0)
    emit_load(0)
    nc.sync.dma_start(wt[:], w_d[:])
    nc.sync.dma_start(pt[:], p_d[:])
    emit_load(1)
    emit_sign(0)

    def conv_mm(cp, spv, xpv, t, c, cc, start, stop):
        """One tap MM for chunk c. Parity cc swaps psum halves."""
        for i in range(2):          # i: img A/B (rhs partition half)
            rp = slice(64 * i, 64 * i + 64)
            ob = 64 * ((i + cc) % 2)  # psum partition half (parity swap)
            op = slice(ob, ob + 64)
            if t < 9:
                ky, kx = divmod(t, 3)
                rhs = spv[rp, ky + 14 * c: ky + 14 * c + 13: 2,
                          1 + kx: 1 + kx + 111: 2]
                w = wt[rp, 64 * t:64 * t + 64]
            else:
                dy, dx = divmod(t - 9, 2)
                rhs = xpv[rp, 1 + 14 * c + dy: 1 + 14 * c + dy + 13: 2,
                          dx: dx + 111: 2]
                w = wt[rp, O_ID:O_ID + 64]
            nc.tensor.matmul(
                cp[op, 512 * cc:512 * cc + CN], w, rhs,
                start=start, stop=stop)

    pending = []   # deferred stage2 emitters from the previous unit

    for k, (p, h) in enumerate(units):
        nA, nB = 2 * p, 2 * p + 1
        oy0 = HALF * h
        xp, _ = xps[k]
        xpv = xp[:].rearrange("p (r c) -> p r c", r=SROWS)
        spv = sp[h][:].rearrange("p (r c) -> p r c", r=SROWS)

        out1 = work.tile([128, UN], f16, tag="out1", name="out1")
        sg2 = work.tile([128, UN], f16, tag="sg2", name="sg2")
        stg = work.tile([128, 2 * UN], f16, tag="stg", name="stg")

        # ---- conv1 + fused avgpool ----
        # units 1+: t-major over the full unit (8 MMs/tap across both psum
        # tiles) so a psum region is revisited every 8 MMs and the
        # accumulation RAW drain hides (185ns/4-MM group); unit 0 runs
        # q-split so its first taps only need slab rows <29.  stage2
        # chunks of unit k-1 interleave at tap boundaries.
        cp_list = [psum.tile([128, 1024], f32, tag="ps", name=f"ps{k}_{q}")
                   for q in range(2)]

        def prelu1_dve(q):
            # 3-op DVE prelu: first op frees the conv psum tile in ~940ns
            # without queueing behind the busy ACT engine
            hs = slice(2 * CN * q, 2 * CN * (q + 1))
            pin = cp_list[q][:].rearrange("p (i n) -> p i n", i=2)[:, :, 0:CN]
            t16 = work.tile([128, 2 * CN], f16, tag="t16", name="t16")
            tv = t16[:].rearrange("p (i n) -> p i n", i=2)
            nc.vector.tensor_scalar(
                tv, pin, s3f, pt[:, PB12:PB12 + 1], ALU.mult, ALU.add)
            nc.vector.tensor_scalar(
                sg2[:, hs].bitcast(u16), t16[:].bitcast(u16),
                0x8000, 0x3C00, ALU.bitwise_and, ALU.bitwise_or)
            m16 = work.tile([128, 2 * CN], f16, tag="m16", name="m16")
            nc.vector.tensor_scalar(
                m16[:], t16[:], 0.0, pt[:, PA1M:PA1M + 1],
                ALU.min, ALU.mult)
            nc.vector.tensor_tensor(
                out1[:, hs], t16[:], m16[:], ALU.add)

        def prelu1_act(q):
            hs = slice(2 * CN * q, 2 * CN * (q + 1))
            pin = cp_list[q][:].rearrange("p (i n) -> p i n", i=2)[:, :, 0:CN]
            pout = out1[:, hs].rearrange("p (i n) -> p i n", i=2)
            nc.scalar.activation(
                pout, pin, AF.Prelu,
                bias=pt[:, PB12:PB12 + 1], scale=s3f,
                alpha=pt[:, PA1:PA1 + 1])
            if has_b13:
                nc.vector.tensor_scalar(
                    out1[:, hs], out1[:, hs], pt[:, PB13:PB13 + 1],
                    None, ALU.add)
            if trick_sign2:
                nc.vector.tensor_scalar(
                    sg2[:, hs].bitcast(u16), out1[:, hs].bitcast(u16),
                    0x8000, 0x3C00, ALU.bitwise_and, ALU.bitwise_or)
            else:
                nc.scalar.activation(
                    sg2[:, hs], out1[:, hs], AF.Sign,
                    bias=pt[:, PBS2:PBS2 + 1])

        dve_ok = trick_sign2 and not has_b13
        for q in range(2):
            for t in range(13):
                for cc in range(2):
                    conv_mm(cp_list[q], spv, xpv, t, 2 * q + cc, cc,
                            start=(t == 0), stop=(t == 12))
                if t in (4, 9) and pending:
                    pending.pop(0)()
            if q == 1 and dve_ok:
                prelu1_dve(1)
            else:
                prelu1_act(q)
            if q == 0:
                emit_sign(k + 1)

        while pending:
            pending.pop(0)()

        # ---- stage 2 (deferred into unit k+1's conv window) ----
        def mk_stage2(c, k=k, out1=out1, sg2=sg2, stg=stg,
                      nA=nA, nB=nB, oy0=oy0):
            def emit(pool2=psum2, tag2="ps2"):
                cs = slice(CN * c, CN * (c + 1))
                p2 = pool2.tile([128, 1024], f32, tag=tag2, name="ps2")
                # slot 0 (cols 0:CN) = img A, slot 1 (512:) = img B;
                # chunk parity decides which sbuf partition half holds A
                for blk, src, st, sp_ in ((O_PW, sg2, True, False),
                                          (O_DIAG, out1, False, True)):
                    for i in range(2):       # i: img A/B (psum slot)
                        rb = 64 * ((i + c) % 2)
                        rp = slice(rb, rb + 64)
                        nc.tensor.matmul(
                            p2[:, 512 * i:512 * i + CN],
                            wt[rp, blk:blk + 128], src[rp, cs],
                            start=st, stop=sp_)
                pin = p2[:].rearrange("p (i n) -> p i n", i=2)[:, :, 0:CN]
                pout = stg[:].rearrange("p (i n) -> p i n", i=2)[:, :, cs]
                nc.scalar.activation(
                    pout, pin, AF.Prelu,
                    bias=pt[:, PB22F:PB22F + 1],
                    scale=pt[:, PS2V:PS2V + 1],
                    alpha=pt[:, PA2F:PA2F + 1])
                if has_b23 and c == NCHUNK - 1:
                    nc.vector.tensor_scalar(
                        stg[:], stg[:], pt[:, PB23F:PB23F + 1],
                        None, ALU.add)
                if has_b23:
                    rr = (0, HALF) if c == NCHUNK - 1 else None
                else:
                    rr = {1: (0, 14), NCHUNK - 1: (14, HALF)}.get(c)
                if rr is not None:
                    # one DMA stores both images (pair-major y_d layout)
                    sv = stg[:].rearrange("p (i n) -> p i n", i=2)
                    nc.sync.dma_start(
                        y_d[nA // 2, :, :,
                            OW * (oy0 + rr[0]):OW * (oy0 + rr[1])],
                        sv[:, :, OW * rr[0]:OW * rr[1]])
            return emit

        pending = [mk_stage2(c) for c in range(NCHUNK)]

    # tail: the conv psum pool is idle now, so alternate the last unit's
    # stage2 chunks between both psum pools — no prelu2 WAR turnaround
    for ci, fn in enumerate(pending):
        if ci % 2 == 1:
            fn(pool2=psum, tag2="ps")
        else:
            fn()
    pending = []

    for cm in reversed(pools):
        cm.__exit__(None, None, None)
    dram_cm.__exit__(None, None, None)
    tc_cm.__exit__(None, None, None)
    nc.compile()
    return nc, x_d.name, w_d.name, p_d.name, y_d.name


def _prep(inputs):
    f32 = np.float32
    f16 = np.float16
    w3 = np.asarray(inputs["w3"], f32)
    wpw1 = np.asarray(inputs["wpw1"], f32)
    wpw2 = np.asarray(inputs["wpw2"], f32)
    a1 = np.asarray(inputs["a1"], f32).reshape(CIN)
    a2 = np.asarray(inputs["a2"], f32).reshape(COUT)
    b11 = np.asarray(inputs["b11"], f32).reshape(CIN)
    b12 = np.asarray(inputs["b12"], f32).reshape(CIN)
    b13 = np.asarray(inputs["b13"], f32).reshape(CIN)
    b21 = np.asarray(inputs["b21"], f32).reshape(CIN)
    b22 = np.asarray(inputs["b22"], f32).reshape(COUT)
    b23 = np.asarray(inputs["b23"], f32).reshape(COUT)

    s3 = f32(np.mean(np.abs(w3))) or f32(1.0)
    s1 = f32(np.mean(np.abs(wpw1))) or f32(1.0)
    s2 = f32(np.mean(np.abs(wpw2))) or f32(1.0)

    d0 = f16(1.0 / (4.0 * float(s3)))
    d1 = f16(1.0 / float(s1))
    d2 = f16(1.0 / float(s2))

    whalf = np.zeros((64, WCOLS), f32)
    sgn = np.sign
    for t in range(9):
        ky, kx = divmod(t, 3)
        whalf[:, 64 * t:64 * t + 64] = sgn(w3[:, :, ky, kx]).T
    whalf[:, O_ID:O_ID + 64] = float(d0) * np.eye(64, dtype=f32)
    whalf[:, O_PW:O_PW + 64] = sgn(wpw1[:, :, 0, 0]).T
    whalf[:, O_PW + 64:O_PW + 128] = sgn(wpw2[:, :, 0, 0]).T
    whalf[:, O_DIAG:O_DIAG + 64] = float(d1) * np.eye(64, dtype=f32)
    whalf[:, O_DIAG + 64:O_DIAG + 128] = float(d2) * np.eye(64, dtype=f32)
    wfull = np.concatenate([whalf, whalf], axis=0).astype(f16)

    def pairc(v):  # channel vec (64,) -> pair-layout (128,)
        return np.concatenate([v, v])

    params = np.zeros((128, NPARAM), f32)
    params[:, PA1] = pairc(a1)
    params[:, PB12] = pairc(b12)
    params[:, PB11] = pairc(b11)
    params[:, PA2F] = a2
    params[:, PB22F] = b22
    params[:, PS2V] = np.concatenate(
        [np.full(64, 1.0 / float(d1), f32), np.full(64, 1.0 / float(d2), f32)])
    params[:, PBS2] = pairc(b13 + b21)
    params[:, PB13] = pairc(b13)
    params[:, PB23F] = b23
    params[:, PA1M] = pairc(a1) - 1.0

    scal = {
        "s3": float(s3),
        "b11_zero": bool(np.all(b11 == 0.0)),
        "trick_sign2": bool(np.all(b13 + b21 == 0.0) and np.all(a1 > 0)),
        "has_b13": bool(np.any(b13 != 0.0)),
        "has_b23": bool(np.any(b23 != 0.0)),
    }
    return wfull, params, scal


def _pack_x(x):
    """x (32,64,112,112) fp32 -> per-core slabs
    [NCORES][NPAIR, 2, 128, 57*112] fp16 (row -1 zero-padded for h=0)."""
    xh = x.astype(np.float16)
    # keep the sign of values that underflow to 0 in fp16 (sign1 must match)
    m = (xh == 0) & (x != 0)
    if m.any():
        xh[m] = np.copysign(np.float16(6e-8), x[m]).astype(np.float16)
    out = np.zeros((NCORES, NPAIR, 2, 2, CIN, SROWS, W), np.float16)
    xc = xh.reshape(NCORES, NPAIR, 2, CIN, H, W)
    for h in range(2):
        r0 = 2 * (HALF * h) - 1
        a = max(r0, 0)
        b = r0 + SROWS
        out[:, :, h, :, :, a - r0:, :] = xc[:, :, :, :, a:b, :]
    # [core, pair, h, img, cin, r, w] -> [core, pair, h, (img cin), r*w]
    return np.ascontiguousarray(
        out.transpose(0, 1, 2, 3, 4, 5, 6)).reshape(
            NCORES, NPAIR, 2, 128, SROWS * W)


def make_in_maps(inputs):
    x = np.asarray(inputs["x"], np.float32)
    wfull, params, scal = _prep(inputs)
    xs = _pack_x(x)
    key = tuple(sorted(scal.items())) + (float(params.sum()),)
    if key not in _cache:
        _cache.clear()
        _cache[key] = _build(scal)
    nc, xn, wn, pn, yn = _cache[key]
    in_maps = [{xn: np.ascontiguousarray(xs[i]), wn: wfull, pn: params}
               for i in range(NCORES)]
    return nc, in_maps, yn


def kernel(**inputs):
    nc, in_maps, yn = make_in_maps(inputs)
    res = bass_utils.run_bass_kernel_spmd(
        nc, in_maps, core_ids=list(range(NCORES)))
    # device output is [pair, ch, img-in-pair, pix]; restore [img, ch, h, w]
    out = np.concatenate(
        [res.results[i][yn].reshape(NPAIR, COUT, 2, OH, OW)
         .transpose(0, 2, 1, 3, 4).reshape(BPC, COUT, OH, OW)
         for i in range(NCORES)], axis=0)
    return out.astype(np.float32)
